# revision 2
# baseline (speedup 1.0000x reference)
"""Distributed Trainium2 kernel for LayerNorm + multi-head self-attention + out-proj.

Reference model (dims hardcoded):
  x [2, 2048, 1024] -> LayerNorm(gamma, beta) -> QKV (w_qkv [1024, 3072])
  -> 16-head attention (d_head 64, scale 1/8) -> out proj (w_out [1024,1024] + b_out)

Sharding (8 NeuronCores): pure head tensor-parallelism. Core g owns global heads
{2g, 2g+1} and processes BOTH batches (tokens flattened to [4096, 1024]).
After attention, a per-head AllToAll redistributes so core g holds all 1024
inner dims for flat token rows [g*512, (g+1)*512); out projection is local.

v2 design (vs the f32r/PE-transpose baseline):
- LayerNorm applied on-chip BEFORE the QKV matmul: bn_stats (DVE) + a fused
  Identity activation (ScalarE) with per-token scale=1/std, bias=-mu/std,
  emitting x_hat directly in fp16. gamma folded into W host-side.
- x_hat^T reaches SBUF via SBUF->SBUF DMA xbar transposes (fp16, 14ns/xbar
  tile) -- zero TensorEngine transpose cost, zero DVE cast cost.
- All matmuls fp16 x fp16 (1 cycle/row, fast weight load); PSUM stays f32.
- Attention interleaved at m-tile granularity: scores of call i+1 issue
  between PV accumulation steps of call i, so the PE never waits on the
  ScalarE exp chain and the clock stays ramped.
- Softmax denominators via a ones-column in the PV stationary; reciprocals
  batched per call on [2,512] (not broadcast-first), then broadcast by a
  K=1 matmul.
- Head loop outermost: AllToAll(h=0) overlaps all of head 1's attention.
  a2a payload fp16, Shared-address outputs.
"""
import numpy as np

import concourse.bass as bass
import concourse.mybir as mybir
import concourse.tile as tile
from concourse import bacc
from concourse.bass_utils import run_bass_kernel_spmd

F32 = mybir.dt.float32
F32R = mybir.dt.float32r
F16 = mybir.dt.float16
AF = mybir.ActivationFunctionType
OP = mybir.AluOpType

B = 2
N = 2048
D = 1024
DH = 64
SCALE = 0.125
EPS = 1e-5

NT = B * N              # 4096 flat tokens
P = 128
NTILES = NT // P        # 32 token tiles
NBLK = NT // 512        # 8 token blocks of 512
DC = D // P             # 8 contraction chunks
H_LOC = 2               # heads per core
QKV_COLS = 3 * H_LOC * DH   # 384 local qkv cols
TOK_OUT = NT // 8       # 512 output rows per core


DEBUG = False


def _build(with_qkv_bias):
    nc = bacc.Bacc("TRN2", target_bir_lowering=False, debug=False, num_devices=8)

    x_ext = nc.dram_tensor("x", [NT, D], F32, kind="ExternalInput")
    wqkv_ext = nc.dram_tensor("wqkv", [D, QKV_COLS], F16, kind="ExternalInput")
    bqkv_ext = nc.dram_tensor("bqkv", [QKV_COLS, 1], F32, kind="ExternalInput")
    wout_ext = nc.dram_tensor("wout", [D, D], F16, kind="ExternalInput")
    bout_ext = nc.dram_tensor("bout", [1, D], F32, kind="ExternalInput")
    out_ext = nc.dram_tensor("out", [TOK_OUT, D], F32, kind="ExternalOutput")
    if DEBUG:
        qT_ext = nc.dram_tensor("qT_d", [P, NT], F16, kind="ExternalOutput")
        kT_ext = nc.dram_tensor("kT_d", [P, NT], F16, kind="ExternalOutput")
        vaug_ext = nc.dram_tensor("vaug_d", [P, NTILES * H_LOC * P], F16,
                                  kind="ExternalOutput")
        a2ai_ext = nc.dram_tensor("a2ai_d", [H_LOC, 8, DH, 512], F16,
                                  kind="ExternalOutput")
        a2ao_ext = nc.dram_tensor("a2ao_d", [H_LOC, 8, DH, 512], F16,
                                  kind="ExternalOutput")

    with tile.TileContext(nc) as tc:
        with tc.tile_pool(name="persist", bufs=1) as pp, \
             tc.tile_pool(name="xs", bufs=6) as xsp, \
             tc.tile_pool(name="xh", bufs=4) as xhp, \
             tc.tile_pool(name="xnt", bufs=16) as xntp, \
             tc.tile_pool(name="es", bufs=20) as esp, \
             tc.tile_pool(name="sans", bufs=4) as sanp, \
             tc.tile_pool(name="small", bufs=4) as smp, \
             tc.tile_pool(name="dram", bufs=1, space="DRAM") as dram, \
             tc.tile_pool(name="ps_s", bufs=2, space="PSUM") as ps_s, \
             tc.tile_pool(name="ps_sa", bufs=2, space="PSUM") as ps_sa, \
             tc.tile_pool(name="ps_m", bufs=2, space="PSUM") as ps_m:

            # ---- constants / weights -------------------------------------
            ones512_32 = pp.tile([1, 512], F32, tag="ones512_32")
            nc.vector.memset(ones512_32[:], 1.0)
            # all-ones [128,64]: rows 0 / 64 serve as K=1 broadcast stationaries
            ones_bc32 = pp.tile([P, 64], F32, tag="ones_bc32")
            nc.vector.memset(ones_bc32[:], 1.0)
            ones_bc = pp.tile([P, 64], F32R, tag="ones_bc")
            nc.vector.tensor_copy(ones_bc[:], ones_bc32[:])
            ones_col128 = pp.tile([1, 128], F32R, tag="ones_col128")
            nc.vector.tensor_copy(ones_col128[:], ones512_32[:, 0:128])
            epsp = pp.tile([P, 1], F32, tag="epsp")
            nc.vector.memset(epsp[:], EPS)

            wqkv = []
            for c in range(DC):
                t = pp.tile([P, QKV_COLS], F16, tag=f"wqkv{c}")
                nc.gpsimd.dma_start(t[:], wqkv_ext.ap()[c * P:(c + 1) * P, :])
                wqkv.append(t)
            if with_qkv_bias:
                bq = pp.tile([P, 1], F32, tag="bq")
                bk = pp.tile([P, 1], F32, tag="bk")
                bv = pp.tile([P, 1], F32, tag="bv")
                nc.sync.dma_start(bq[:], bqkv_ext.ap()[0:P, :])
                nc.sync.dma_start(bk[:], bqkv_ext.ap()[P:2 * P, :])
                nc.sync.dma_start(bv[:], bqkv_ext.ap()[2 * P:3 * P, :])
                qkv_bias = {0: bq, 1: bk, 2: bv}
            # out-proj weights, preloaded early (fp16)
            wo = []
            for c in range(DC):
                t = pp.tile([P, D], F16, tag=f"wout{c}")
                nc.gpsimd.dma_start(t[:], wout_ext.ap()[c * P:(c + 1) * P, :])
                wo.append(t)
            bout = pp.tile([1, D], F32R, tag="bout")
            nc.gpsimd.dma_start(bout[:], bout_ext.ap())
            bout_bc = pp.tile([P, D], F32, tag="bout_bc")
            for half in range(2):
                bb = ps_m.tile([P, 512], F32, tag="m", name=f"bbp_{half}")
                nc.tensor.matmul(bb[:], ones_col128[:],
                                 bout[0:1, half * 512:(half + 1) * 512],
                                 start=True, stop=True)
                nc.vector.tensor_copy(bout_bc[:, half * 512:(half + 1) * 512], bb[:])

            # persistent activations
            qT = pp.tile([P, NT], F16, tag="qT")     # parts h*64.. = head h
            kT = pp.tile([P, NT], F16, tag="kT")
            vaug = pp.tile([P, NTILES, H_LOC, P], F16, tag="vaug")
            ones64_32 = pp.tile([P, NTILES * H_LOC], F32, tag="ones64_32")
            nc.vector.memset(ones64_32[:], 1.0)
            nc.vector.tensor_copy(
                vaug[:, :, :, DH:DH + 1].rearrange("p a b c -> p (a b c)"),
                ones64_32[:])  # slot padded to 128 cols for xbar alignment

            a2a_in = [dram.tile([8, DH, 512], F16, name=f"a2a_in{h}", tag=f"a2a_in{h}")
                      for h in range(H_LOC)]
            a2a_out = [dram.tile([8, DH, 512], F16, name=f"a2a_out{h}",
                                 tag=f"a2a_out{h}")
                       for h in range(H_LOC)]

            # ---- phase 1: LayerNorm + x_hat^T + QKV per 512-block --------
            def qkv_block(blk):
                xhTs = []
                for t in range(4):
                    i = blk * 4 + t
                    xt = xsp.tile([P, D], F32, tag="x", bufs=6, name=f"x_{i}")
                    nc.gpsimd.dma_start(xt[:], x_ext.ap()[i * P:(i + 1) * P, :])
                    stats = smp.tile([P, 2, 6], F32, tag="stats", name=f"st_{i}")
                    nc.vector.bn_stats(stats[:, 0, :], xt[:, 0:512])
                    nc.vector.bn_stats(stats[:, 1, :], xt[:, 512:1024])
                    mv = smp.tile([P, 2], F32, tag="mv", name=f"mv_{i}")
                    nc.vector.bn_aggr(mv[:], stats[:])
                    sd = smp.tile([P, 1], F32, tag="sd", name=f"sd_{i}")
                    nc.scalar.activation(sd[:], mv[:, 1:2], AF.Sqrt, bias=epsp[:])
                    rstd = smp.tile([P, 1], F32, tag="rstd", name=f"rstd_{i}")
                    nc.vector.reciprocal(rstd[:], sd[:])
                    negmu = smp.tile([P, 1], F32, tag="negmu", name=f"nm_{i}")
                    nc.vector.tensor_tensor(negmu[:], mv[:, 0:1], rstd[:], OP.mult)
                    nc.vector.tensor_scalar_mul(negmu[:], negmu[:], -1.0)
                    xh = xhp.tile([P, D], F16, tag="xh", bufs=4, name=f"xh_{i}")
                    nc.scalar.activation(xh[:], xt[:], AF.Identity,
                                         bias=negmu[:], scale=rstd[:])
                    xhTs.append(xh)
                xT = xntp.tile([P, DC, 512], F16, tag="xnt", bufs=2,
                               name=f"xT_{blk}")
                for t in range(4):
                    nc.sync.dma_start_transpose(
                        xT[:, :, t * P:(t + 1) * P], xhTs[t][:])

                vtb = xntp.tile([P, 512], F16, tag="vtb", bufs=2, name=f"vtb_{blk}")
                for grp, dst, col in ((0, qT, blk * 512), (1, kT, blk * 512),
                                      (2, vtb, 0)):
                    acc = ps_m.tile([P, 512], F32, tag="m", name=f"qkv_{blk}_{grp}")
                    for c in range(DC):
                        nc.tensor.matmul(acc[:], wqkv[c][:, grp * P:(grp + 1) * P],
                                         xT[:, c, :], start=(c == 0), stop=(c == DC - 1))
                    if with_qkv_bias:
                        nc.vector.tensor_scalar(dst[:, col:col + 512], acc[:],
                                                qkv_bias[grp][:], None, OP.add)
                    else:
                        nc.vector.tensor_copy(dst[:, col:col + 512], acc[:])
                # v token-major via batched per-head DMA xbar transposes
                for h in range(H_LOC):
                    nc.sync.dma_start_transpose(
                        vaug[:, blk * 4:(blk + 1) * 4, h, 0:DH],
                        vtb[h * DH:(h + 1) * DH, :])

            for blk in range(NBLK):
                qkv_block(blk)

            if DEBUG:
                nc.sync.dma_start(qT_ext.ap(), qT[:])
                nc.sync.dma_start(kT_ext.ap(), kT[:])
                nc.sync.dma_start(
                    vaug_ext.ap(),
                    vaug[:].rearrange("p a b c -> p (a b c)"))

            # ---- phase 2: attention, m-interleaved across calls ----------
            # call order: head outermost so a2a(h=0) overlaps head 1 compute
            calls = [(h, b, tq) for h in range(H_LOC) for b in range(B)
                     for tq in range(2)]

            def scores_step(i, m):
                h, b, tq = calls[i]
                hp = h * DH
                q0 = b * N + tq * 1024
                mt = b * 16 + m
                s = ps_s.tile([P, 1024], F32, tag="s", name=f"s_{i}_{m}")
                for hf in range(2):
                    nc.tensor.matmul(
                        s[:, hf * 512:(hf + 1) * 512],
                        kT[hp:hp + DH, mt * P:(mt + 1) * P],
                        qT[hp:hp + DH, q0 + hf * 512:q0 + (hf + 1) * 512],
                        start=True, stop=True)
                e = esp.tile([P, 1024], F16, tag="e", name=f"e_{i}_{m}")
                nc.scalar.activation(e[:], s[:], AF.Exp, bias=0.0, scale=SCALE)
                return e

            pv_banks = {}

            def pv_step(i, m, es):
                h, b, tq = calls[i]
                mt = b * 16 + m
                if m == 0:
                    pv_banks[i] = [
                        ps_sa.tile([DH + 1, 512], F32, tag="sa",
                                   name=f"sa_{i}_{hf}")
                        for hf in range(2)]
                for hf in range(2):
                    nc.tensor.matmul(
                        pv_banks[i][hf],
                        vaug[:, mt, h, 0:DH + 1],
                        es[m][:, hf * 512:(hf + 1) * 512],
                        start=(m == 0), stop=(m == 15))

            def norm_evict(i):
                h, b, tq = calls[i]
                sa = pv_banks.pop(i)
                z = smp.tile([P, 512], F32, tag="z", bufs=2, name=f"z_{i}")
                for hf in range(2):
                    nc.vector.tensor_copy(z[hf * DH:hf * DH + 1, :],
                                          sa[hf][DH:DH + 1, :])
                zi = smp.tile([P, 512], F32R, tag="zi", bufs=2, name=f"zi_{i}")
                with nc.allow_low_precision(reason="f32r is bit-identical to f32"):
                    nc.vector.reciprocal(zi[0:DH + 1, :], z[0:DH + 1, :])
                for hf in range(2):
                    zb = ps_m.tile([P, 512], F32, tag="m", name=f"zb_{i}_{hf}")
                    nc.tensor.matmul(zb[0:DH, :],
                                     ones_bc[hf * DH:hf * DH + 1, :],
                                     zi[hf * DH:hf * DH + 1, :],
                                     start=True, stop=True)
                    zbc = sanp.tile([DH, 512], F32, tag="zbc", bufs=2,
                                    name=f"zbc_{i}_{hf}")
                    nc.vector.tensor_copy(zbc[:], zb[0:DH, :])
                    saN = sanp.tile([DH, 512], F16, tag="saN", bufs=4,
                                    name=f"saN_{i}_{hf}")
                    nc.vector.tensor_mul(saN[:], sa[hf][0:DH, :], zbc[:])
                    j = b * 4 + tq * 2 + hf
                    nc.sync.dma_start(a2a_in[h][j, :, :], saN[:])

            es_by_call = {}
            for step in range(9):
                if step < 8:
                    h, b, tq = calls[step]
                    es_by_call[step] = []
                    for m in range(16):
                        es_by_call[step].append(scores_step(step, m))
                        if step >= 1:
                            pv_step(step - 1, m, es_by_call[step - 1])
                else:
                    for m in range(16):
                        pv_step(step - 1, m, es_by_call[step - 1])
                if step >= 1:
                    norm_evict(step - 1)
                    del es_by_call[step - 1]
                    if step - 1 == 3:
                        nc.gpsimd.collective_compute(
                            "AllToAll", OP.bypass,
                            replica_groups=[[0, 1, 2, 3, 4, 5, 6, 7]],
                            ins=[a2a_in[0].opt()],
                            outs=[a2a_out[0].opt()],
                        )
                    if step - 1 == 7:
                        nc.gpsimd.collective_compute(
                            "AllToAll", OP.bypass,
                            replica_groups=[[0, 1, 2, 3, 4, 5, 6, 7]],
                            ins=[a2a_in[1].opt()],
                            outs=[a2a_out[1].opt()],
                        )

            if DEBUG:
                for h in range(H_LOC):
                    nc.sync.dma_start(a2ai_ext.ap()[h], a2a_in[h][:])
                    nc.sync.dma_start(a2ao_ext.ap()[h], a2a_out[h][:])

            # ---- phase 3: local out-projection ---------------------------
            xa = []
            for c in range(DC):
                t = xntp.tile([P, 512], F16, tag="xa", bufs=8, name=f"xa_{c}")
                nc.gpsimd.dma_start(t[0:DH, :], a2a_out[0][c, :, :])
                nc.gpsimd.dma_start(t[DH:P, :], a2a_out[1][c, :, :])
                xa.append(t)
            for t in range(4):
                accs = [ps_m.tile([P, 512], F32, tag="m", name=f"op_{t}_{half}")
                        for half in range(2)]
                for c in range(DC):
                    for half in range(2):
                        nc.tensor.matmul(accs[half], xa[c][:, t * P:(t + 1) * P],
                                         wo[c][:, half * 512:(half + 1) * 512],
                                         start=(c == 0), stop=(c == DC - 1))
                for half in range(2):
                    ot = sanp.tile([P, 512], F32, tag="ot", bufs=2,
                                   name=f"ot_{t}_{half}")
                    nc.vector.tensor_add(ot[:], accs[half],
                                         bout_bc[:, half * 512:(half + 1) * 512])
                    nc.sync.dma_start(
                        out_ext.ap()[t * P:(t + 1) * P, half * 512:(half + 1) * 512],
                        ot[:])

    nc.compile()
    return nc


_NC_CACHE = {}
_last_in_maps = None


def kernel(x, gamma, beta, w_qkv, w_out, b_out):
    x = np.ascontiguousarray(np.asarray(x, dtype=np.float32).reshape(NT, D))
    gamma = np.asarray(gamma, dtype=np.float32)
    beta = np.asarray(beta, dtype=np.float32)
    w_qkv = np.asarray(w_qkv, dtype=np.float32)
    w_out = np.asarray(w_out, dtype=np.float32)
    b_out = np.asarray(b_out, dtype=np.float32)

    # fold LayerNorm's affine (gamma, beta) into the QKV projection
    w_eff = gamma[:, None] * w_qkv            # [1024, 3072]
    b_eff = beta @ w_qkv                      # [3072]
    with_bias = bool(np.any(b_eff != 0.0))

    if with_bias not in _NC_CACHE:
        _NC_CACHE[with_bias] = _build(with_bias)
    nc = _NC_CACHE[with_bias]

    in_maps = []
    for g in range(8):
        cols = []
        for part in range(3):                 # q, k, v column slices of heads {2g, 2g+1}
            c0 = part * D + g * (H_LOC * DH)
            cols.append(np.arange(c0, c0 + H_LOC * DH))
        cols = np.concatenate(cols)
        in_maps.append({
            "x": x,
            "wqkv": np.ascontiguousarray(w_eff[:, cols]).astype(np.float16),
            "bqkv": np.ascontiguousarray(b_eff[cols][:, None]),
            "wout": w_out.astype(np.float16),
            "bout": np.ascontiguousarray(b_out[None, :]),
        })

    global _last_in_maps
    _last_in_maps = in_maps
    res = run_bass_kernel_spmd(nc, in_maps, core_ids=list(range(8)))
    out = np.empty((NT, D), dtype=np.float32)
    for g in range(8):
        out[g * TOK_OUT:(g + 1) * TOK_OUT, :] = res.results[g]["out"]
    return out.reshape(B, N, D)
